# revision 88
# baseline (speedup 1.0000x reference)
"""Windowed multi-head attention TRN2 kernel (Bass/Tile), SPMD over 8 cores.

Problem (per reference): x:(8,512,64,64) viewed as (B, 4096 tok, 512 c);
Q/K/V = tok @ W^T + b; per window (64 tok) & head (8 x 64d):
softmax(QK^T/8 + Bbias) @ V; output back in (B,512,64,64).

Sharding: data-parallel, one batch element per core (8 cores).

Per-core dataflow (all matmuls fp16 operands, fp32 PSUM accum):
 - host passes x^T (c, tok) fp16 so projection rhs tiles DMA contiguously
 - Q^T,K^T computed in [c_out, tok] layout (heads pairs on partition halves)
 - V computed in natural [tok, c] layout, with a per-head ones-column
   appended (65-wide head blocks) so PV matmuls also produce softmax
   denominators
 - scores^T = K^T_wh^T-matmul: [k,q] tiles packed 8 units/PSUM bank,
   head-parity (e) on partition halves
 - softmax without max-subtraction (scores are O(1)): exp on ACT; the
   exp(Bbias^T) elementwise multiply on DVE simultaneously moves probs
   to window-parity (p) partition halves, so PV runs against natural V
   (no duplicate) and outputs land in natural token rows
 - PV: [64q, 65] units, 4 per PSUM bank; normalize via one batched
   reciprocal + one batched multiply per bank during PSUM->SBUF evac
"""

import sys
import numpy as np

for _p in ("/opt/trn_rl_repo",):
    if _p not in sys.path:
        sys.path.insert(0, _p)

from contextlib import ExitStack

import concourse.bass as bass
import concourse.tile as tile
from concourse import mybir

F16 = mybir.dt.float16
F32 = mybir.dt.float32

B, C, HH, WW = 8, 512, 64, 64
NH, HD = 8, 64
WIN = 64            # tokens per window
TOK = C * 0 + 4096  # tokens per batch/core
NT = 8              # 512-token tiles per core
NCHUNK = 4          # 128-channel chunks

TRACE = False
LAST = {}
SCORES_BD = True  # scores via block-diagonal K (full 128-part contraction)
PIPELINE = 1      # emit attention this many T-tiles behind projections
STORE_SP = True   # issue output stores from SP instead of ACT
PROJBUFS = 3      # PSUM banks for projection groups
OPSBUFS = 1       # PSUM buffers per PV output tag (2 tags)
SPSBUFS = 1       # PSUM buffers per scores tag
ATTSPLIT = 1      # split exp/prob-mul for finer attention wavefront
PMAJOR = 1        # scores matmuls in window-parity-major order
ONBUFS = 2        # SBUF buffers for output staging tiles (2 tags)
SSPLIT = 1        # separate scores PSUM tile per window parity
S0DOUBLE = 1      # double-buffer parity-0 scores (uses the spare bank)
EBUFS = 2         # SBUF buffers for attention et/pt/on tiles
INTERLEAVE = 4    # attention subtiles inside their own T, after V groups


def _emit(tc, out, xT, wq, wk, wv, ebt, bqk, iters=1, parts="pas"):
    """Emit the per-core program. bqk: [128, 8] fp32 (bq/8 | bk chunks) or None.
    parts: subset of 'p' (projections), 'a' (attention), 's' (store) for
    timing ablations."""
    nc = tc.nc
    Exp = mybir.ActivationFunctionType.Exp
    Ident = mybir.ActivationFunctionType.Identity

    with ExitStack() as ctx:
        ep = ctx.enter_context

        tbufs = PIPELINE + 1
        wpool = ep(tc.tile_pool(name="w", bufs=1))
        xpool = ep(tc.tile_pool(name="x", bufs=tbufs))
        qkpool = ep(tc.tile_pool(name="qk", bufs=tbufs))
        vpool = ep(tc.tile_pool(name="v", bufs=tbufs))
        epool = ep(tc.tile_pool(name="e", bufs=EBUFS))
        bdpool = ep(tc.tile_pool(name="bd", bufs=tbufs))
        rcpool = ep(tc.tile_pool(name="rc", bufs=4))
        onpool = ep(tc.tile_pool(name="on", bufs=ONBUFS))
        projps = ep(tc.tile_pool(name="projps", bufs=PROJBUFS, space="PSUM"))
        sps = ep(tc.tile_pool(name="sps", bufs=SPSBUFS, space="PSUM"))
        # one PSUM bank is spare: optionally double-buffer parity-0 scores
        sps0 = (ep(tc.tile_pool(name="sps0", bufs=2, space="PSUM"))
                if S0DOUBLE and SSPLIT else sps)
        ops = ep(tc.tile_pool(name="ops", bufs=OPSBUFS, space="PSUM"))

        # resident weights: [c_in chunk 128, c_out 512] fp16 per proj
        wsb = {}
        for nm, wdram in (("q", wq), ("k", wk), ("v", wv)):
            for ci in range(NCHUNK):
                t = wpool.tile([128, 512], F16, tag=f"w{nm}{ci}")
                nc.sync.dma_start(t[:], wdram[ci * 128:(ci + 1) * 128, :])
                wsb[nm, ci] = t
        ebt_sb = wpool.tile([128, 64], F16, tag="ebt")
        nc.sync.dma_start(ebt_sb[:], ebt[:, :])
        bqk_sb = None
        if bqk is not None:
            bqk_sb = wpool.tile([128, 8], F32, tag="bqk")
            nc.sync.dma_start(bqk_sb[:], bqk[:, :])

        # ---- one-time inits: zero blocks of block-diagonal tiles and the
        # ones-columns of V survive every iteration (later writes only
        # touch the data blocks), so initialize all rotating buffers here,
        # outside the timing loop. Tile calls advance each tag's rotation
        # by bufs=2, preserving in-loop phase.
        for i in range(max(tbufs, EBUFS)):
            if SCORES_BD and i < tbufs:
                for j in range(4):
                    t = bdpool.tile([128, 1024], F16, tag=f"bdk{j}")
                    nc.gpsimd.memset(t[:], 0)
            if i < EBUFS:
                t = epool.tile([128, 1024], F16, tag="pt")
                nc.gpsimd.memset(t[:], 0)
            if i < tbufs:
                for tt in range(NCHUNK):
                    vn = vpool.tile([128, 520], F16, tag=f"vn{tt}")
                    nc.scalar.activation(
                        vn[:].rearrange("p (h x) -> p h x", x=65)[:, :, 64],
                        ebt_sb[:, 0:8], Ident, bias=1.0, scale=0.0)

        if iters > 1:
            # on-device repetition for timing: amortizes host dispatch
            ep(tc.For_i(0, iters))

        def emit_proj_group(nm, co, xt, dst, bdk=None):
            """One projection PSUM group (4 matmuls) + its evacuation."""
            pi = 0 if nm == "q" else 1
            ps = projps.tile([128, 512], F32, tag="proj")
            if nm == "v":
                for ci in range(NCHUNK):
                    nc.tensor.matmul(
                        ps[:],
                        xt[ci][:, co * 128:(co + 1) * 128],
                        wsb["v", ci][:],
                        start=(ci == 0), stop=(ci == NCHUNK - 1))
                if "P" not in parts:
                    nc.vector.tensor_copy(
                        dst[:].rearrange("p (h x) -> p h x", x=65)
                        [:, :, 0:64],
                        ps[:].rearrange("p (h x) -> p h x", x=64))
                return
            for ci in range(NCHUNK):
                w_ap = (wsb[nm, 0][:, 0:128] if "W" in parts
                        else wsb[nm, ci][:, co * 128:(co + 1) * 128])
                if "A" in parts:  # ablation: independent matmuls, no accum
                    nc.tensor.matmul(
                        ps[:], w_ap, xt[0 if "W" in parts else ci][:],
                        start=True, stop=True)
                else:
                    nc.tensor.matmul(
                        ps[:], w_ap, xt[0 if "W" in parts else ci][:],
                        start=(ci == 0), stop=(ci == NCHUNK - 1))
            if "P" in parts:
                return
            if bdk is not None:
                # K straight into block-diagonal layout, window-major:
                # col g*128 + e*64 + k, g = 2tt+p
                bd_v = bdk[:].rearrange("r (g c) -> r g c", c=128)
                ps_v = ps[:].rearrange("r (g k) -> r g k", k=64)
                for e in range(2):
                    re = slice(e * 64, e * 64 + 64)
                    if bqk_sb is not None:
                        nc.scalar.activation(
                            bd_v[re, :, e * 64:e * 64 + 64],
                            ps_v[re], Ident,
                            bias=bqk_sb[re, 4 + co:5 + co])
                    else:
                        nc.scalar.copy(
                            bd_v[re, :, e * 64:e * 64 + 64], ps_v[re])
            elif bqk_sb is not None:
                nc.scalar.activation(
                    dst[:], ps[:], Ident,
                    bias=bqk_sb[:, pi * 4 + co:pi * 4 + co + 1])
            else:
                nc.scalar.copy(dst[:], ps[:])

        def proj_thunks(T):
            """xt loads (immediate) + 12 emission thunks for T's
            projection PSUM groups; returns (thunks, state_entry)."""
            xt = []
            for ci in range(NCHUNK):
                t = xpool.tile([128, 512], F16, tag=f"xt{ci}")
                nc.sync.dma_start(
                    t[:],
                    xT[ci * 128:(ci + 1) * 128, T * 512:(T + 1) * 512])
                xt.append(t)
            qkt = {}
            bdks = []
            vnat = []
            thunks = []

            def emit_proj_pair(nm, co0, xt):
                # ablation: interleave two groups' accumulation chains so
                # consecutive matmuls never hit the same PSUM region
                ps0 = projps.tile([128, 512], F32, tag="proj")
                ps1 = projps.tile([128, 512], F32, tag="proj")
                for ci in range(NCHUNK):
                    for k, ps in ((0, ps0), (1, ps1)):
                        co = co0 + k
                        nc.tensor.matmul(
                            ps[:],
                            wsb[nm, ci][:, co * 128:(co + 1) * 128],
                            xt[ci][:],
                            start=(ci == 0), stop=(ci == NCHUNK - 1))

            names = ("q",) if SCORES_BD else ("q", "k")
            for nm in names:
                if "I" in parts and "p" in parts:
                    for co0 in (0, 2):
                        thunks.append(
                            lambda nm=nm, co0=co0: emit_proj_pair(
                                nm, co0, xt))
                    for co in range(NCHUNK):
                        t = qkpool.tile([128, 512], F16, tag=f"{nm}t{co}")
                        qkt[nm, co] = t
                    continue
                for co in range(NCHUNK):
                    t = qkpool.tile([128, 512], F16, tag=f"{nm}t{co}")
                    qkt[nm, co] = t
                    if "p" in parts:
                        thunks.append(
                            lambda nm=nm, co=co, t=t: emit_proj_group(
                                nm, co, xt, t))
            if SCORES_BD:
                for j in range(4):
                    bdk = bdpool.tile([128, 1024], F16, tag=f"bdk{j}")
                    bdks.append(bdk)
                    if "p" in parts:
                        thunks.append(
                            lambda j=j, bdk=bdk: emit_proj_group(
                                "k", j, xt, None, bdk=bdk))
            for tt in range(NCHUNK):
                vn = vpool.tile([128, 520], F16, tag=f"vn{tt}")
                vnat.append(vn)
                if "p" in parts:
                    thunks.append(
                        lambda tt=tt, vn=vn: emit_proj_group(
                            "v", tt, xt, vn))
            return thunks, (qkt, bdks, vnat)

        def emit_attn_scores(qkt, bdks, Ta, tt):
            # ---- attention: subtile tt covers windows 2tt, 2tt+1 of Ta.
            # HAZARD RULE: concurrent matmuls with disjoint row-groups but
            # a shared column-group collide in the PE array (device crash);
            # sub-128 matmuls are placed DIAGONALLY (out partition base ==
            # operand partition base). Scores land head-parity packed (e on
            # halves); the exp(Bbias)-multiply on DVE moves probs to
            # block-diagonal window-parity layout, so PV runs full-width
            # against natural V and outputs land in natural token rows.
            if True:
                if SSPLIT and SCORES_BD:
                    # separate PSUM tile per window parity so exp of parity
                    # p depends on only its own 4 matmuls even with
                    # tile-granular PSUM dependency tracking
                    s2 = []
                    for p in range(2):
                        s_half = (sps0 if p == 0 else sps).tile(
                            [128, 256], F32, tag=f"s{p}")
                        s2.append(s_half)
                    for p in range(2):
                        for j in range(4):
                            w = 2 * tt + p
                            nc.tensor.matmul(
                                s2[p][:, j * 64:(j + 1) * 64],
                                bdks[j][:, tt * 256 + p * 128:
                                        tt * 256 + (p + 1) * 128],
                                qkt["q", j][:, w * 64:(w + 1) * 64],
                                start=True, stop=True)
                    s = None
                else:
                    s = sps.tile([128, 512], F32, tag="s")
                if SCORES_BD and not SSPLIT:
                    # p-major: with ATTSPLIT, exp of parity p waits only on
                    # its own 4 matmuls
                    for p, j in (
                            [(p, j) for p in range(2) for j in range(4)]
                            if PMAJOR else
                            [(p, j) for j in range(4) for p in range(2)]):
                        if True:
                            w = 2 * tt + p
                            nc.tensor.matmul(
                                s[:, (j * 2 + p) * 64:(j * 2 + p + 1) * 64],
                                bdks[j][:, tt * 256 + p * 128:
                                        tt * 256 + (p + 1) * 128],
                                qkt["q", j][:, w * 64:(w + 1) * 64],
                                start=True, stop=True)
                elif not SCORES_BD:
                    for j in range(4):
                        for e in range(2):
                            r = slice(e * 64, e * 64 + 64)
                            for p in range(2):
                                w = 2 * tt + p
                                wc = slice(w * 64, w * 64 + 64)
                                nc.tensor.matmul(
                                    s[r, (j * 2 + p) * 64:
                                      (j * 2 + p + 1) * 64],
                                    qkt["k", j][r, wc],
                                    qkt["q", j][r, wc],
                                    start=True, stop=True)
                et = epool.tile([128, 512], F16, tag="et")
                et_v = et[:].rearrange("r (j u q) -> r j u q", u=2, q=64)
                if SSPLIT and SCORES_BD:
                    for p in range(2):
                        nc.scalar.activation(
                            et_v[:, :, p, :],
                            s2[p][:].rearrange("r (j q) -> r j q", q=64),
                            Exp)
                elif ATTSPLIT:
                    # split exp per window parity: the (p, e) multiplies
                    # depend only on their own exp half
                    s_v = s[:].rearrange("r (j u q) -> r j u q", u=2, q=64)
                    for p in range(2):
                        nc.scalar.activation(
                            et_v[:, :, p, :], s_v[:, :, p, :], Exp)
                else:
                    nc.scalar.activation(et[:], s[:], Exp)
                # block-diagonal probs: pt[p*64+k, h*128+p*64+q] =
                # et[e*64+k, (2j+p)*64+q]*ebt[k,q] (h=2j+e); off-diagonal
                # blocks stay zero, so one PV matmul covers both windows
                # with full 128-partition contraction against natural V.
                pt = epool.tile([128, 1024], F16, tag="pt")
                pt_v = pt[:].rearrange("r (j z) -> r j z", j=4)
                for p in range(2):
                    rp = slice(p * 64, p * 64 + 64)
                    for e in range(2):
                        re = slice(e * 64, e * 64 + 64)
                        c0 = e * 128 + p * 64
                        if ATTSPLIT:
                            # split per bank-half: PV bank b waits only on
                            # the j in {2b, 2b+1} multiplies
                            for bh in range(2):
                                js = slice(bh * 2, bh * 2 + 2)
                                nc.vector.tensor_mul(
                                    pt_v[rp, js, c0:c0 + 64],
                                    et_v[re, js, p, :],
                                    ebt_sb[re, 0:64].unsqueeze(1)
                                    .broadcast_to((64, 2, 64)))
                        else:
                            nc.vector.tensor_mul(
                                pt_v[rp, :, c0:c0 + 64],
                                et_v[re, :, p, :],
                                ebt_sb[re, 0:64].unsqueeze(1)
                                .broadcast_to((64, 4, 64)))
                return pt

        def emit_attn_pv(pt, vnat, Ta, tt):
            if True:
                # PV: 8 matmuls (one per head), full 128 partitions; two
                # PSUM banks of 4 [128q2w, 65] units each.
                on = onpool.tile([128, 512], F32, tag=f"on{tt % 2}")
                for b in range(2):
                    o = ops.tile([128, 260], F32, tag=f"ob{b}")
                    o_v = o[:].rearrange("r (u x) -> r u x", x=65)
                    for u in range(4):
                        h = 4 * b + u
                        nc.tensor.matmul(
                            o[:, u * 65:(u + 1) * 65],
                            pt[:, h * 128:(h + 1) * 128],
                            vnat[tt][:, h * 65:(h + 1) * 65],
                            start=True, stop=True)
                    rc = rcpool.tile([128, 4], F32, tag=f"rc{b}")
                    nc.vector.reciprocal(rc[:, 0:4], o_v[:, :, 64])
                    nc.vector.tensor_mul(
                        on[:].rearrange("r (b2 u q) -> r b2 u q", b2=2, q=64)
                        [:, b, :, :],
                        o_v[:, :, 0:64],
                        rc[:, 0:4].unsqueeze(2).broadcast_to((128, 4, 64)))
                if "s" in parts:
                    eng = nc.sync if STORE_SP else nc.scalar
                    eng.dma_start(
                        out[Ta * 512 + tt * 128: Ta * 512 + (tt + 1) * 128,
                            :],
                        on[:])

        # software pipeline driver: attention trails projections by
        # PIPELINE T-tiles; with INTERLEAVE, attention subtiles are emitted
        # between projection groups as scheduler priority hints.
        state = {}
        for T in range(NT + PIPELINE):
            pthunks = []
            if T < NT:
                pthunks, entry = proj_thunks(T)
                state[T] = entry
            def attn_closures(Ta):
                q_, b_, v_ = state[Ta]
                cls = []
                for tt in range(NCHUNK):
                    cell = {}

                    def a_sc(tt=tt, q=q_, bb=b_, Ta=Ta, cell=cell):
                        cell["pt"] = emit_attn_scores(q, bb, Ta, tt)

                    def a_pv(tt=tt, v=v_, Ta=Ta, cell=cell):
                        emit_attn_pv(cell["pt"], v, Ta, tt)

                    if INTERLEAVE == 2:
                        cls.append((a_sc, a_pv))
                    else:
                        cls.append((lambda a=a_sc, b=a_pv: (a(), b()),))
                return cls

            if INTERLEAVE == 4 and "a" in parts:
                # all four subtiles inside their own T, each right after
                # the V-projection group it needs: no cross-T tail at all
                cls = list(attn_closures(T)) if T < NT else []
                for i, th in enumerate(pthunks):
                    th()
                    if i >= 8 and cls:
                        cls.pop(0)[0]()
                for tup in cls:
                    tup[0]()
                state.pop(T, None)
                continue

            if INTERLEAVE == 3 and "a" in parts:
                # quarter-T shift: tts 1-3 of T-1 go in proj(T) slots
                # {2,5,8}; tt 0 of T goes right after its first V group
                # (slot 8), shrinking the un-overlapped attention tail.
                if T < NT:
                    state[T] = (state[T][0], state[T][1], state[T][2],
                                attn_closures(T))
                athunks = list(state[T - 1][3][1:4]) if T >= 1 else []
                for i, th in enumerate(pthunks):
                    th()
                    if i in (2, 5, 7) and athunks:
                        athunks.pop(0)[0]()
                    if i == 8 and T < NT:
                        state[T][3][0][0]()
                for th in athunks:
                    th[0]()
                if T >= 1:
                    state.pop(T - 1)
                continue

            athunks = []
            if T >= PIPELINE and "a" in parts:
                Ta = T - PIPELINE
                athunks = [t for t in attn_closures(Ta)]
                state.pop(Ta)
            if INTERLEAVE and pthunks and athunks:
                for i, th in enumerate(pthunks):
                    th()
                    if INTERLEAVE == 2:
                        if i % 3 != 0 and athunks:
                            a, b = athunks.pop(0)
                            a(), b()
                    elif i % 3 == 2 and athunks:
                        athunks.pop(0)[0]()
            else:
                for th in pthunks:
                    th()
            for tup in athunks:
                for f in tup:
                    f()


def _legalize_sync(nc, max_waits=1):
    """Hoist excess semaphore waits into standalone same-engine
    EventSemaphore instructions. Engine instruction streams execute in
    order, so a wait carried by an immediately-preceding EventSemaphore is
    equivalent to a wait on the instruction itself — and the walrus build
    in this environment rejects instructions with more than one wait."""
    import bass_rust
    n_new = 0
    fn = nc.m.functions[0]
    for blk in fn.blocks:
        out = []
        changed = False
        for ins in blk.instructions:
            si = ins.sync_info
            waits = list(si.on_wait) if si and si.on_wait else []
            if len(waits) > max_waits:
                keep = waits[-max_waits:]
                for w in waits[:-max_waits]:
                    es = mybir.InstEventSemaphore(
                        name=f"esw-{n_new}-{ins.name}", ins=[], outs=[])
                    es.engine = ins.engine
                    es.sync_info = bass_rust.SyncInfo(on_wait=[w], on_update=[])
                    out.append(es)
                    n_new += 1
                ins.sync_info = bass_rust.SyncInfo(
                    on_wait=keep,
                    on_update=list(si.on_update) if si.on_update else [])
                changed = True
            out.append(ins)
        if changed:
            blk.instructions = out
    return n_new


def _build_model(with_bias, iters=1, parts="pas"):
    nc = bass.Bass("TRN2", target_bir_lowering=False, debug=False,
                   enable_partition_id=False)
    xT = nc.dram_tensor("xT", [512, 4096], F16, kind="ExternalInput").ap()
    wq = nc.dram_tensor("wq", [512, 512], F16, kind="ExternalInput").ap()
    wk = nc.dram_tensor("wk", [512, 512], F16, kind="ExternalInput").ap()
    wv = nc.dram_tensor("wv", [512, 512], F16, kind="ExternalInput").ap()
    ebt = nc.dram_tensor("ebt", [128, 64], F16, kind="ExternalInput").ap()
    bqk = (nc.dram_tensor("bqk", [128, 8], F32, kind="ExternalInput").ap()
           if with_bias else None)
    out = nc.dram_tensor("out", [4096, 512], F32, kind="ExternalOutput").ap()
    with tile.TileContext(nc) as tc:
        _emit(tc, out, xT, wq, wk, wv, ebt, bqk, iters=iters, parts=parts)
    return nc


_MODEL_CACHE = {}


def get_model(with_bias=False, legalize=True, iters=1, parts="pas"):
    key = (with_bias, legalize, iters, parts, SCORES_BD, PIPELINE, STORE_SP,
           PROJBUFS, OPSBUFS, EBUFS, INTERLEAVE, SPSBUFS, ATTSPLIT, PMAJOR,
           ONBUFS, SSPLIT, S0DOUBLE)
    if key not in _MODEL_CACHE:
        nc = _build_model(with_bias, iters=iters, parts=parts)
        if legalize:
            _legalize_sync(nc)
        _MODEL_CACHE[key] = nc
    return _MODEL_CACHE[key]


def make_in_maps(x, Wq, bq, Wk, bk, Wv, bv, Bbias):
    """Host-side sharding + layout prep. Returns (in_maps, with_bias)."""
    x = np.asarray(x, np.float32)
    with_bias = bool(np.any(bq) or np.any(bk))
    if np.any(bv):
        raise NotImplementedError("nonzero bv not supported")
    wq16 = np.ascontiguousarray(np.asarray(Wq, np.float32).T / 8.0).astype(np.float16)
    wk16 = np.ascontiguousarray(np.asarray(Wk, np.float32).T).astype(np.float16)
    wv16 = np.ascontiguousarray(np.asarray(Wv, np.float32).T).astype(np.float16)
    eb = np.exp(np.asarray(Bbias, np.float32).T)
    ebt = np.concatenate([eb, eb], 0).astype(np.float16)  # [128 (k x2), 64 q]
    common = {"wq": wq16, "wk": wk16, "wv": wv16, "ebt": ebt}
    if with_bias:
        bqk = np.concatenate(
            [np.asarray(bq, np.float32).reshape(4, 128).T / 8.0,
             np.asarray(bk, np.float32).reshape(4, 128).T], 1)  # [128, 8]
        common["bqk"] = np.ascontiguousarray(bqk)
    in_maps = []
    for b in range(B):
        xT16 = np.ascontiguousarray(
            x[b].reshape(TOK, C).T).astype(np.float16)
        in_maps.append({"xT": xT16, **common})
    return in_maps, with_bias


def kernel(**inputs):
    from concourse.bass_utils import run_bass_kernel_spmd
    in_maps, with_bias = make_in_maps(**inputs)
    nc = get_model(with_bias)
    res = run_bass_kernel_spmd(
        nc, in_maps, core_ids=list(range(B)), trace=TRACE)
    LAST["results"] = res
    out = np.stack([r["out"] for r in res.results], 0)
    return out.reshape(B, C, HH, WW)


def _harvest_io(nc):
    import jax
    in_names, out_names, out_avals = [], [], []
    for alloc in nc.m.functions[0].allocations:
        if not isinstance(alloc, mybir.MemoryLocationSet):
            continue
        name = alloc.memorylocations[0].name
        if alloc.kind == "ExternalInput":
            in_names.append(name)
        elif alloc.kind == "ExternalOutput":
            out_names.append(name)
            out_avals.append(jax.core.ShapedArray(
                tuple(alloc.tensor_shape), mybir.dt.np(alloc.dtype)))
    return in_names, out_names, out_avals


def _make_timed_callable(nc, in_maps):
    """Build a jitted shard_map callable around the single bass_exec of
    `nc` (mirrors run_bass_via_pjrt, but with NO donation so the same
    device-resident args can be reused across timed calls; outputs are
    garbage — timing only). Returns a zero-arg closure that runs one
    dispatch and blocks."""
    import jax
    from jax.sharding import Mesh, PartitionSpec
    from jax.experimental.shard_map import shard_map
    from concourse import bass2jax

    bass2jax.install_neuronx_cc_hook()
    in_names, out_names, out_avals = _harvest_io(nc)
    n_params = len(in_names)
    all_names = tuple(in_names + out_names)
    n_cores = len(in_maps)

    def _body(*args):
        return tuple(bass2jax._bass_exec_p.bind(
            *args,
            out_avals=tuple(out_avals),
            in_names=all_names,
            out_names=tuple(out_names),
            lowering_input_output_aliases=(),
            sim_require_finite=True,
            sim_require_nnan=True,
            nc=nc))

    devices = jax.devices()[:n_cores]
    mesh = Mesh(np.asarray(devices), ("core",))
    n_all = n_params + len(out_names)
    sharded = jax.jit(shard_map(
        _body, mesh=mesh,
        in_specs=(PartitionSpec("core"),) * n_all,
        out_specs=(PartitionSpec("core"),) * len(out_names),
        check_rep=False), keep_unused=True)
    concat_in = [
        np.concatenate([np.asarray(m[name]) for m in in_maps], 0)
        for name in in_names]
    concat_zeros = [
        np.zeros((n_cores * a.shape[0], *a.shape[1:]), a.dtype)
        for a in out_avals]
    args = [jax.device_put(a) for a in concat_in + concat_zeros]
    jax.block_until_ready(sharded(*args))  # warm-up / compile

    def run():
        jax.block_until_ready(sharded(*args))
    return run


def time_kernel(inputs, iters=4096, samples=8, parts="pas"):
    """Returns ns per iteration. Builds two model variants — the body run
    once vs `1+iters` times inside an on-device For_i loop — and
    differences median wall-clock over `samples` dispatches of each. With
    ~1s on-device per N-iter dispatch, the ~±20ms axon dispatch jitter
    contributes <2% error."""
    import time
    in_maps, with_bias = make_in_maps(**inputs)
    run1 = _make_timed_callable(
        get_model(with_bias, iters=1, parts=parts), in_maps)
    runN = _make_timed_callable(
        get_model(with_bias, iters=1 + iters, parts=parts), in_maps)
    t1s, tNs = [], []
    for _ in range(samples):
        t0 = time.time(); run1(); t1s.append(time.time() - t0)
        t0 = time.time(); runN(); tNs.append(time.time() - t0)
    t1 = float(np.median(t1s)); tN = float(np.median(tNs))
    return (tN - t1) / iters * 1e9, (t1s, tNs)



# revision 95
# speedup vs baseline: 1.1965x; 1.1965x over previous
"""Windowed multi-head attention TRN2 kernel (Bass/Tile), SPMD over 8 cores.

Problem (per reference): x:(8,512,64,64) viewed as (B, 4096 tok, 512 c);
Q/K/V = tok @ W^T + b; per window (64 tok) & head (8 x 64d):
softmax(QK^T/8 + Bbias) @ V; output back in (B,512,64,64).

Sharding: data-parallel, one batch element per core (8 cores).

Per-core dataflow (all matmuls fp16 operands, fp32 PSUM accum):
 - host passes x^T (c, tok) fp16 so projection rhs tiles DMA contiguously
 - Q^T,K^T computed in [c_out, tok] layout (heads pairs on partition halves)
 - V computed in natural [tok, c] layout, with a per-head ones-column
   appended (65-wide head blocks) so PV matmuls also produce softmax
   denominators
 - scores^T = K^T_wh^T-matmul: [k,q] tiles packed 8 units/PSUM bank,
   head-parity (e) on partition halves
 - softmax without max-subtraction (scores are O(1)): exp on ACT; the
   exp(Bbias^T) elementwise multiply on DVE simultaneously moves probs
   to window-parity (p) partition halves, so PV runs against natural V
   (no duplicate) and outputs land in natural token rows
 - PV: [64q, 65] units, 4 per PSUM bank; normalize via one batched
   reciprocal + one batched multiply per bank during PSUM->SBUF evac
"""

import sys
import numpy as np

for _p in ("/opt/trn_rl_repo",):
    if _p not in sys.path:
        sys.path.insert(0, _p)

from contextlib import ExitStack

import concourse.bass as bass
import concourse.tile as tile
from concourse import mybir

F16 = mybir.dt.float16
F32 = mybir.dt.float32

B, C, HH, WW = 8, 512, 64, 64
NH, HD = 8, 64
WIN = 64            # tokens per window
TOK = C * 0 + 4096  # tokens per batch/core
NT = 8              # 512-token tiles per core
NCHUNK = 4          # 128-channel chunks

TRACE = False
LAST = {}
SCORES_BD = True  # scores via block-diagonal K (full 128-part contraction)
PIPELINE = 1      # emit attention this many T-tiles behind projections
STORE_SP = True   # issue output stores from SP instead of ACT
PROJBUFS = 3      # PSUM banks for projection groups
OPSBUFS = 1       # PSUM buffers per PV output tag (2 tags)
SPSBUFS = 1       # PSUM buffers per scores tag
ATTSPLIT = 1      # split exp/prob-mul for finer attention wavefront
PMAJOR = 1        # scores matmuls in window-parity-major order
ONBUFS = 2        # SBUF buffers for output staging tiles (2 tags)
SSPLIT = 1        # separate scores PSUM tile per window parity
S0DOUBLE = 1      # double-buffer parity-0 scores (uses the spare bank)
QEVAC_POOL = 0    # Q-projection evacuation on gpsimd instead of ACT
KEVAC_DVE = 0     # K block-diagonal evacuation on DVE instead of ACT
EBUFS = 2         # SBUF buffers for attention et/pt/on tiles
INTERLEAVE = 3    # quarter-T-shifted attention interleave (see driver)


def _emit(tc, out, xT, wq, wk, wv, ebt, bqk, iters=1, parts="pas"):
    """Emit the per-core program. bqk: [128, 8] fp32 (bq/8 | bk chunks) or None.
    parts: subset of 'p' (projections), 'a' (attention), 's' (store) for
    timing ablations."""
    nc = tc.nc
    Exp = mybir.ActivationFunctionType.Exp
    Ident = mybir.ActivationFunctionType.Identity

    with ExitStack() as ctx:
        ep = ctx.enter_context

        tbufs = PIPELINE + 1
        wpool = ep(tc.tile_pool(name="w", bufs=1))
        xpool = ep(tc.tile_pool(name="x", bufs=tbufs))
        qkpool = ep(tc.tile_pool(name="qk", bufs=tbufs))
        vpool = ep(tc.tile_pool(name="v", bufs=tbufs))
        epool = ep(tc.tile_pool(name="e", bufs=EBUFS))
        bdpool = ep(tc.tile_pool(name="bd", bufs=tbufs))
        rcpool = ep(tc.tile_pool(name="rc", bufs=4))
        onpool = ep(tc.tile_pool(name="on", bufs=ONBUFS))
        projps = ep(tc.tile_pool(name="projps", bufs=PROJBUFS, space="PSUM"))
        sps = ep(tc.tile_pool(name="sps", bufs=SPSBUFS, space="PSUM"))
        # one PSUM bank is spare: optionally double-buffer parity-0 scores
        sps0 = (ep(tc.tile_pool(name="sps0", bufs=2, space="PSUM"))
                if S0DOUBLE and SSPLIT else sps)
        ops = ep(tc.tile_pool(name="ops", bufs=OPSBUFS, space="PSUM"))

        # resident weights: [c_in chunk 128, c_out 512] fp16 per proj
        wsb = {}
        for nm, wdram in (("q", wq), ("k", wk), ("v", wv)):
            for ci in range(NCHUNK):
                t = wpool.tile([128, 512], F16, tag=f"w{nm}{ci}")
                nc.sync.dma_start(t[:], wdram[ci * 128:(ci + 1) * 128, :])
                wsb[nm, ci] = t
        ebt_sb = wpool.tile([128, 64], F16, tag="ebt")
        nc.sync.dma_start(ebt_sb[:], ebt[:, :])
        bqk_sb = None
        if bqk is not None:
            bqk_sb = wpool.tile([128, 8], F32, tag="bqk")
            nc.sync.dma_start(bqk_sb[:], bqk[:, :])

        # ---- one-time inits: zero blocks of block-diagonal tiles and the
        # ones-columns of V survive every iteration (later writes only
        # touch the data blocks), so initialize all rotating buffers here,
        # outside the timing loop. Tile calls advance each tag's rotation
        # by bufs=2, preserving in-loop phase.
        for i in range(max(tbufs, EBUFS)):
            if SCORES_BD and i < tbufs:
                for j in range(4):
                    t = bdpool.tile([128, 1024], F16, tag=f"bdk{j}")
                    nc.gpsimd.memset(t[:], 0)
            if i < EBUFS:
                t = epool.tile([128, 1024], F16, tag="pt")
                nc.gpsimd.memset(t[:], 0)
            if i < tbufs:
                for tt in range(NCHUNK):
                    vn = vpool.tile([128, 520], F16, tag=f"vn{tt}")
                    nc.scalar.activation(
                        vn[:].rearrange("p (h x) -> p h x", x=65)[:, :, 64],
                        ebt_sb[:, 0:8], Ident, bias=1.0, scale=0.0)

        if iters > 1:
            # on-device repetition for timing: amortizes host dispatch
            ep(tc.For_i(0, iters))

        def emit_proj_group(nm, co, xt, dst, bdk=None):
            """One projection PSUM group (4 matmuls) + its evacuation."""
            pi = 0 if nm == "q" else 1
            ps = projps.tile([128, 512], F32, tag="proj")
            if nm == "v":
                for ci in range(NCHUNK):
                    nc.tensor.matmul(
                        ps[:],
                        xt[ci][:, co * 128:(co + 1) * 128],
                        wsb["v", ci][:],
                        start=(ci == 0), stop=(ci == NCHUNK - 1))
                if "P" not in parts:
                    nc.vector.tensor_copy(
                        dst[:].rearrange("p (h x) -> p h x", x=65)
                        [:, :, 0:64],
                        ps[:].rearrange("p (h x) -> p h x", x=64))
                return
            for ci in range(NCHUNK):
                w_ap = (wsb[nm, 0][:, 0:128] if "W" in parts
                        else wsb[nm, ci][:, co * 128:(co + 1) * 128])
                if "A" in parts:  # ablation: independent matmuls, no accum
                    nc.tensor.matmul(
                        ps[:], w_ap, xt[0 if "W" in parts else ci][:],
                        start=True, stop=True)
                else:
                    nc.tensor.matmul(
                        ps[:], w_ap, xt[0 if "W" in parts else ci][:],
                        start=(ci == 0), stop=(ci == NCHUNK - 1))
            if "P" in parts:
                return
            if bdk is not None:
                # K straight into block-diagonal layout, window-major:
                # col g*128 + e*64 + k, g = 2tt+p
                bd_v = bdk[:].rearrange("r (g c) -> r g c", c=128)
                ps_v = ps[:].rearrange("r (g k) -> r g k", k=64)
                for e in range(2):
                    re = slice(e * 64, e * 64 + 64)
                    if bqk_sb is not None:
                        nc.scalar.activation(
                            bd_v[re, :, e * 64:e * 64 + 64],
                            ps_v[re], Ident,
                            bias=bqk_sb[re, 4 + co:5 + co])
                    elif KEVAC_DVE:
                        # unclog ACT's queue ahead of exp
                        nc.vector.tensor_copy(
                            bd_v[re, :, e * 64:e * 64 + 64], ps_v[re])
                    else:
                        nc.scalar.copy(
                            bd_v[re, :, e * 64:e * 64 + 64], ps_v[re])
            elif bqk_sb is not None:
                nc.scalar.activation(
                    dst[:], ps[:], Ident,
                    bias=bqk_sb[:, pi * 4 + co:pi * 4 + co + 1])
            elif QEVAC_POOL:
                # unclog ACT (whose queue delays exp): Q evac on gpsimd
                nc.gpsimd.tensor_copy(dst[:], ps[:])
            else:
                nc.scalar.copy(dst[:], ps[:])

        def proj_thunks(T):
            """xt loads (immediate) + 12 emission thunks for T's
            projection PSUM groups; returns (thunks, state_entry)."""
            xt = []
            for ci in range(NCHUNK):
                t = xpool.tile([128, 512], F16, tag=f"xt{ci}")
                nc.sync.dma_start(
                    t[:],
                    xT[ci * 128:(ci + 1) * 128, T * 512:(T + 1) * 512])
                xt.append(t)
            qkt = {}
            bdks = []
            vnat = []
            thunks = []

            def emit_proj_pair(nm, co0, xt):
                # ablation: interleave two groups' accumulation chains so
                # consecutive matmuls never hit the same PSUM region
                ps0 = projps.tile([128, 512], F32, tag="proj")
                ps1 = projps.tile([128, 512], F32, tag="proj")
                for ci in range(NCHUNK):
                    for k, ps in ((0, ps0), (1, ps1)):
                        co = co0 + k
                        nc.tensor.matmul(
                            ps[:],
                            wsb[nm, ci][:, co * 128:(co + 1) * 128],
                            xt[ci][:],
                            start=(ci == 0), stop=(ci == NCHUNK - 1))

            names = ("q",) if SCORES_BD else ("q", "k")
            for nm in names:
                if "I" in parts and "p" in parts:
                    for co0 in (0, 2):
                        thunks.append(
                            lambda nm=nm, co0=co0: emit_proj_pair(
                                nm, co0, xt))
                    for co in range(NCHUNK):
                        t = qkpool.tile([128, 512], F16, tag=f"{nm}t{co}")
                        qkt[nm, co] = t
                    continue
                for co in range(NCHUNK):
                    t = qkpool.tile([128, 512], F16, tag=f"{nm}t{co}")
                    qkt[nm, co] = t
                    if "p" in parts:
                        thunks.append(
                            lambda nm=nm, co=co, t=t: emit_proj_group(
                                nm, co, xt, t))
            if SCORES_BD:
                for j in range(4):
                    bdk = bdpool.tile([128, 1024], F16, tag=f"bdk{j}")
                    bdks.append(bdk)
                    if "p" in parts:
                        thunks.append(
                            lambda j=j, bdk=bdk: emit_proj_group(
                                "k", j, xt, None, bdk=bdk))
            for tt in range(NCHUNK):
                vn = vpool.tile([128, 520], F16, tag=f"vn{tt}")
                vnat.append(vn)
                if "p" in parts:
                    thunks.append(
                        lambda tt=tt, vn=vn: emit_proj_group(
                            "v", tt, xt, vn))
            return thunks, (qkt, bdks, vnat)

        def emit_attn_scores(qkt, bdks, Ta, tt):
            # ---- attention: subtile tt covers windows 2tt, 2tt+1 of Ta.
            # HAZARD RULE: concurrent matmuls with disjoint row-groups but
            # a shared column-group collide in the PE array (device crash);
            # sub-128 matmuls are placed DIAGONALLY (out partition base ==
            # operand partition base). Scores land head-parity packed (e on
            # halves); the exp(Bbias)-multiply on DVE moves probs to
            # block-diagonal window-parity layout, so PV runs full-width
            # against natural V and outputs land in natural token rows.
            if True:
                if SSPLIT and SCORES_BD:
                    # separate PSUM tile per window parity so exp of parity
                    # p depends on only its own 4 matmuls even with
                    # tile-granular PSUM dependency tracking
                    s2 = []
                    for p in range(2):
                        s_half = (sps0 if p == 0 else sps).tile(
                            [128, 256], F32, tag=f"s{p}")
                        s2.append(s_half)
                    for p in range(2):
                        for j in range(4):
                            w = 2 * tt + p
                            nc.tensor.matmul(
                                s2[p][:, j * 64:(j + 1) * 64],
                                bdks[j][:, tt * 256 + p * 128:
                                        tt * 256 + (p + 1) * 128],
                                qkt["q", j][:, w * 64:(w + 1) * 64],
                                start=True, stop=True)
                    s = None
                else:
                    s = sps.tile([128, 512], F32, tag="s")
                if SCORES_BD and not SSPLIT:
                    # p-major: with ATTSPLIT, exp of parity p waits only on
                    # its own 4 matmuls
                    for p, j in (
                            [(p, j) for p in range(2) for j in range(4)]
                            if PMAJOR else
                            [(p, j) for j in range(4) for p in range(2)]):
                        if True:
                            w = 2 * tt + p
                            nc.tensor.matmul(
                                s[:, (j * 2 + p) * 64:(j * 2 + p + 1) * 64],
                                bdks[j][:, tt * 256 + p * 128:
                                        tt * 256 + (p + 1) * 128],
                                qkt["q", j][:, w * 64:(w + 1) * 64],
                                start=True, stop=True)
                elif not SCORES_BD:
                    for j in range(4):
                        for e in range(2):
                            r = slice(e * 64, e * 64 + 64)
                            for p in range(2):
                                w = 2 * tt + p
                                wc = slice(w * 64, w * 64 + 64)
                                nc.tensor.matmul(
                                    s[r, (j * 2 + p) * 64:
                                      (j * 2 + p + 1) * 64],
                                    qkt["k", j][r, wc],
                                    qkt["q", j][r, wc],
                                    start=True, stop=True)
                et = epool.tile([128, 512], F16, tag="et")
                et_v = et[:].rearrange("r (j u q) -> r j u q", u=2, q=64)
                if SSPLIT and SCORES_BD:
                    for p in range(2):
                        nc.scalar.activation(
                            et_v[:, :, p, :],
                            s2[p][:].rearrange("r (j q) -> r j q", q=64),
                            Exp)
                elif ATTSPLIT:
                    # split exp per window parity: the (p, e) multiplies
                    # depend only on their own exp half
                    s_v = s[:].rearrange("r (j u q) -> r j u q", u=2, q=64)
                    for p in range(2):
                        nc.scalar.activation(
                            et_v[:, :, p, :], s_v[:, :, p, :], Exp)
                else:
                    nc.scalar.activation(et[:], s[:], Exp)
                # block-diagonal probs: pt[p*64+k, h*128+p*64+q] =
                # et[e*64+k, (2j+p)*64+q]*ebt[k,q] (h=2j+e); off-diagonal
                # blocks stay zero, so one PV matmul covers both windows
                # with full 128-partition contraction against natural V.
                pt = epool.tile([128, 1024], F16, tag="pt")
                pt_v = pt[:].rearrange("r (j z) -> r j z", j=4)
                for p in range(2):
                    rp = slice(p * 64, p * 64 + 64)
                    for e in range(2):
                        re = slice(e * 64, e * 64 + 64)
                        c0 = e * 128 + p * 64
                        if ATTSPLIT:
                            # split per bank-half: PV bank b waits only on
                            # the j in {2b, 2b+1} multiplies
                            for bh in range(2):
                                js = slice(bh * 2, bh * 2 + 2)
                                nc.vector.tensor_mul(
                                    pt_v[rp, js, c0:c0 + 64],
                                    et_v[re, js, p, :],
                                    ebt_sb[re, 0:64].unsqueeze(1)
                                    .broadcast_to((64, 2, 64)))
                        else:
                            nc.vector.tensor_mul(
                                pt_v[rp, :, c0:c0 + 64],
                                et_v[re, :, p, :],
                                ebt_sb[re, 0:64].unsqueeze(1)
                                .broadcast_to((64, 4, 64)))
                return pt

        def emit_attn_pv(pt, vnat, Ta, tt):
            if True:
                # PV: 8 matmuls (one per head), full 128 partitions; two
                # PSUM banks of 4 [128q2w, 65] units each.
                on = onpool.tile([128, 512], F32, tag=f"on{tt % 2}")
                for b in range(2):
                    o = ops.tile([128, 260], F32, tag=f"ob{b}")
                    o_v = o[:].rearrange("r (u x) -> r u x", x=65)
                    for u in range(4):
                        h = 4 * b + u
                        nc.tensor.matmul(
                            o[:, u * 65:(u + 1) * 65],
                            pt[:, h * 128:(h + 1) * 128],
                            vnat[tt][:, h * 65:(h + 1) * 65],
                            start=True, stop=True)
                    rc = rcpool.tile([128, 4], F32, tag=f"rc{b}")
                    nc.vector.reciprocal(rc[:, 0:4], o_v[:, :, 64])
                    nc.vector.tensor_mul(
                        on[:].rearrange("r (b2 u q) -> r b2 u q", b2=2, q=64)
                        [:, b, :, :],
                        o_v[:, :, 0:64],
                        rc[:, 0:4].unsqueeze(2).broadcast_to((128, 4, 64)))
                if "s" in parts:
                    eng = nc.sync if STORE_SP else nc.scalar
                    eng.dma_start(
                        out[Ta * 512 + tt * 128: Ta * 512 + (tt + 1) * 128,
                            :],
                        on[:])

        # software pipeline driver: attention trails projections by
        # PIPELINE T-tiles; with INTERLEAVE, attention subtiles are emitted
        # between projection groups as scheduler priority hints.
        state = {}
        for T in range(NT + PIPELINE):
            pthunks = []
            if T < NT:
                pthunks, entry = proj_thunks(T)
                state[T] = entry
            def attn_closures(Ta):
                q_, b_, v_ = state[Ta]
                cls = []
                for tt in range(NCHUNK):
                    cell = {}

                    def a_sc(tt=tt, q=q_, bb=b_, Ta=Ta, cell=cell):
                        cell["pt"] = emit_attn_scores(q, bb, Ta, tt)

                    def a_pv(tt=tt, v=v_, Ta=Ta, cell=cell):
                        emit_attn_pv(cell["pt"], v, Ta, tt)

                    if INTERLEAVE == 2:
                        cls.append((a_sc, a_pv))
                    else:
                        cls.append((lambda a=a_sc, b=a_pv: (a(), b()),))
                return cls

            if INTERLEAVE == 4 and "a" in parts:
                # all four subtiles inside their own T, each right after
                # the V-projection group it needs: no cross-T tail at all
                cls = list(attn_closures(T)) if T < NT else []
                for i, th in enumerate(pthunks):
                    th()
                    if i >= 8 and cls:
                        cls.pop(0)[0]()
                for tup in cls:
                    tup[0]()
                state.pop(T, None)
                continue

            if INTERLEAVE == 3 and "a" in parts:
                # quarter-T shift: tts 1-3 of T-1 go in proj(T) slots
                # {2,5,8}; tt 0 of T goes right after its first V group
                # (slot 8), shrinking the un-overlapped attention tail.
                if T < NT:
                    state[T] = (state[T][0], state[T][1], state[T][2],
                                attn_closures(T))
                athunks = list(state[T - 1][3][1:4]) if T >= 1 else []
                for i, th in enumerate(pthunks):
                    th()
                    if i in (2, 5, 7) and athunks:
                        athunks.pop(0)[0]()
                    if i == 8 and T < NT:
                        state[T][3][0][0]()
                for th in athunks:
                    th[0]()
                if T >= 1:
                    state.pop(T - 1)
                continue

            athunks = []
            if T >= PIPELINE and "a" in parts:
                Ta = T - PIPELINE
                athunks = [t for t in attn_closures(Ta)]
                state.pop(Ta)
            if INTERLEAVE and pthunks and athunks:
                for i, th in enumerate(pthunks):
                    th()
                    if INTERLEAVE == 2:
                        if i % 3 != 0 and athunks:
                            a, b = athunks.pop(0)
                            a(), b()
                    elif i % 3 == 2 and athunks:
                        athunks.pop(0)[0]()
            else:
                for th in pthunks:
                    th()
            for tup in athunks:
                for f in tup:
                    f()


def _legalize_sync(nc, max_waits=1):
    """Hoist excess semaphore waits into standalone same-engine
    EventSemaphore instructions. Engine instruction streams execute in
    order, so a wait carried by an immediately-preceding EventSemaphore is
    equivalent to a wait on the instruction itself — and the walrus build
    in this environment rejects instructions with more than one wait."""
    import bass_rust
    n_new = 0
    fn = nc.m.functions[0]
    for blk in fn.blocks:
        out = []
        changed = False
        for ins in blk.instructions:
            si = ins.sync_info
            waits = list(si.on_wait) if si and si.on_wait else []
            if len(waits) > max_waits:
                keep = waits[-max_waits:]
                for w in waits[:-max_waits]:
                    es = mybir.InstEventSemaphore(
                        name=f"esw-{n_new}-{ins.name}", ins=[], outs=[])
                    es.engine = ins.engine
                    es.sync_info = bass_rust.SyncInfo(on_wait=[w], on_update=[])
                    out.append(es)
                    n_new += 1
                ins.sync_info = bass_rust.SyncInfo(
                    on_wait=keep,
                    on_update=list(si.on_update) if si.on_update else [])
                changed = True
            out.append(ins)
        if changed:
            blk.instructions = out
    return n_new


def _build_model(with_bias, iters=1, parts="pas"):
    nc = bass.Bass("TRN2", target_bir_lowering=False, debug=False,
                   enable_partition_id=False)
    xT = nc.dram_tensor("xT", [512, 4096], F16, kind="ExternalInput").ap()
    wq = nc.dram_tensor("wq", [512, 512], F16, kind="ExternalInput").ap()
    wk = nc.dram_tensor("wk", [512, 512], F16, kind="ExternalInput").ap()
    wv = nc.dram_tensor("wv", [512, 512], F16, kind="ExternalInput").ap()
    ebt = nc.dram_tensor("ebt", [128, 64], F16, kind="ExternalInput").ap()
    bqk = (nc.dram_tensor("bqk", [128, 8], F32, kind="ExternalInput").ap()
           if with_bias else None)
    out = nc.dram_tensor("out", [4096, 512], F32, kind="ExternalOutput").ap()
    with tile.TileContext(nc) as tc:
        _emit(tc, out, xT, wq, wk, wv, ebt, bqk, iters=iters, parts=parts)
    return nc


_MODEL_CACHE = {}


def get_model(with_bias=False, legalize=True, iters=1, parts="pas"):
    key = (with_bias, legalize, iters, parts, SCORES_BD, PIPELINE, STORE_SP,
           PROJBUFS, OPSBUFS, EBUFS, INTERLEAVE, SPSBUFS, ATTSPLIT, PMAJOR,
           ONBUFS, SSPLIT, S0DOUBLE, QEVAC_POOL, KEVAC_DVE)
    if key not in _MODEL_CACHE:
        nc = _build_model(with_bias, iters=iters, parts=parts)
        if legalize:
            _legalize_sync(nc)
        _MODEL_CACHE[key] = nc
    return _MODEL_CACHE[key]


def make_in_maps(x, Wq, bq, Wk, bk, Wv, bv, Bbias):
    """Host-side sharding + layout prep. Returns (in_maps, with_bias)."""
    x = np.asarray(x, np.float32)
    with_bias = bool(np.any(bq) or np.any(bk))
    if np.any(bv):
        raise NotImplementedError("nonzero bv not supported")
    wq16 = np.ascontiguousarray(np.asarray(Wq, np.float32).T / 8.0).astype(np.float16)
    wk16 = np.ascontiguousarray(np.asarray(Wk, np.float32).T).astype(np.float16)
    wv16 = np.ascontiguousarray(np.asarray(Wv, np.float32).T).astype(np.float16)
    eb = np.exp(np.asarray(Bbias, np.float32).T)
    ebt = np.concatenate([eb, eb], 0).astype(np.float16)  # [128 (k x2), 64 q]
    common = {"wq": wq16, "wk": wk16, "wv": wv16, "ebt": ebt}
    if with_bias:
        bqk = np.concatenate(
            [np.asarray(bq, np.float32).reshape(4, 128).T / 8.0,
             np.asarray(bk, np.float32).reshape(4, 128).T], 1)  # [128, 8]
        common["bqk"] = np.ascontiguousarray(bqk)
    in_maps = []
    for b in range(B):
        xT16 = np.ascontiguousarray(
            x[b].reshape(TOK, C).T).astype(np.float16)
        in_maps.append({"xT": xT16, **common})
    return in_maps, with_bias


def kernel(**inputs):
    from concourse.bass_utils import run_bass_kernel_spmd
    in_maps, with_bias = make_in_maps(**inputs)
    nc = get_model(with_bias)
    res = run_bass_kernel_spmd(
        nc, in_maps, core_ids=list(range(B)), trace=TRACE)
    LAST["results"] = res
    out = np.stack([r["out"] for r in res.results], 0)
    return out.reshape(B, C, HH, WW)


def _harvest_io(nc):
    import jax
    in_names, out_names, out_avals = [], [], []
    for alloc in nc.m.functions[0].allocations:
        if not isinstance(alloc, mybir.MemoryLocationSet):
            continue
        name = alloc.memorylocations[0].name
        if alloc.kind == "ExternalInput":
            in_names.append(name)
        elif alloc.kind == "ExternalOutput":
            out_names.append(name)
            out_avals.append(jax.core.ShapedArray(
                tuple(alloc.tensor_shape), mybir.dt.np(alloc.dtype)))
    return in_names, out_names, out_avals


def _make_timed_callable(nc, in_maps):
    """Build a jitted shard_map callable around the single bass_exec of
    `nc` (mirrors run_bass_via_pjrt, but with NO donation so the same
    device-resident args can be reused across timed calls; outputs are
    garbage — timing only). Returns a zero-arg closure that runs one
    dispatch and blocks."""
    import jax
    from jax.sharding import Mesh, PartitionSpec
    from jax.experimental.shard_map import shard_map
    from concourse import bass2jax

    bass2jax.install_neuronx_cc_hook()
    in_names, out_names, out_avals = _harvest_io(nc)
    n_params = len(in_names)
    all_names = tuple(in_names + out_names)
    n_cores = len(in_maps)

    def _body(*args):
        return tuple(bass2jax._bass_exec_p.bind(
            *args,
            out_avals=tuple(out_avals),
            in_names=all_names,
            out_names=tuple(out_names),
            lowering_input_output_aliases=(),
            sim_require_finite=True,
            sim_require_nnan=True,
            nc=nc))

    devices = jax.devices()[:n_cores]
    mesh = Mesh(np.asarray(devices), ("core",))
    n_all = n_params + len(out_names)
    sharded = jax.jit(shard_map(
        _body, mesh=mesh,
        in_specs=(PartitionSpec("core"),) * n_all,
        out_specs=(PartitionSpec("core"),) * len(out_names),
        check_rep=False), keep_unused=True)
    concat_in = [
        np.concatenate([np.asarray(m[name]) for m in in_maps], 0)
        for name in in_names]
    concat_zeros = [
        np.zeros((n_cores * a.shape[0], *a.shape[1:]), a.dtype)
        for a in out_avals]
    args = [jax.device_put(a) for a in concat_in + concat_zeros]
    jax.block_until_ready(sharded(*args))  # warm-up / compile

    def run():
        jax.block_until_ready(sharded(*args))
    return run


def time_kernel(inputs, iters=4096, samples=8, parts="pas"):
    """Returns ns per iteration. Builds two model variants — the body run
    once vs `1+iters` times inside an on-device For_i loop — and
    differences median wall-clock over `samples` dispatches of each. With
    ~1s on-device per N-iter dispatch, the ~±20ms axon dispatch jitter
    contributes <2% error."""
    import time
    in_maps, with_bias = make_in_maps(**inputs)
    run1 = _make_timed_callable(
        get_model(with_bias, iters=1, parts=parts), in_maps)
    runN = _make_timed_callable(
        get_model(with_bias, iters=1 + iters, parts=parts), in_maps)
    t1s, tNs = [], []
    for _ in range(samples):
        t0 = time.time(); run1(); t1s.append(time.time() - t0)
        t0 = time.time(); runN(); tNs.append(time.time() - t0)
    t1 = float(np.median(t1s)); tN = float(np.median(tNs))
    return (tN - t1) / iters * 1e9, (t1s, tNs)



# revision 99
# speedup vs baseline: 1.2128x; 1.0136x over previous
"""Windowed multi-head attention TRN2 kernel (Bass/Tile), SPMD over 8 cores.

Problem (per reference): x:(8,512,64,64) viewed as (B, 4096 tok, 512 c);
Q/K/V = tok @ W^T + b; per window (64 tok) & head (8 x 64d):
softmax(QK^T/8 + Bbias) @ V; output back in (B,512,64,64).

Sharding: data-parallel, one batch element per core (8 cores).

Per-core dataflow (all matmuls fp16 operands, fp32 PSUM accum):
 - host passes x^T (c, tok) fp16 so projection rhs tiles DMA contiguously
 - Q^T,K^T computed in [c_out, tok] layout (heads pairs on partition halves)
 - V computed in natural [tok, c] layout, with a per-head ones-column
   appended (65-wide head blocks) so PV matmuls also produce softmax
   denominators
 - scores^T = K^T_wh^T-matmul: [k,q] tiles packed 8 units/PSUM bank,
   head-parity (e) on partition halves
 - softmax without max-subtraction (scores are O(1)): exp on ACT; the
   exp(Bbias^T) elementwise multiply on DVE simultaneously moves probs
   to window-parity (p) partition halves, so PV runs against natural V
   (no duplicate) and outputs land in natural token rows
 - PV: [64q, 65] units, 4 per PSUM bank; normalize via one batched
   reciprocal + one batched multiply per bank during PSUM->SBUF evac
"""

import sys
import numpy as np

for _p in ("/opt/trn_rl_repo",):
    if _p not in sys.path:
        sys.path.insert(0, _p)

from contextlib import ExitStack

import concourse.bass as bass
import concourse.tile as tile
from concourse import mybir

F16 = mybir.dt.float16
F32 = mybir.dt.float32

B, C, HH, WW = 8, 512, 64, 64
NH, HD = 8, 64
WIN = 64            # tokens per window
TOK = C * 0 + 4096  # tokens per batch/core
NT = 8              # 512-token tiles per core
NCHUNK = 4          # 128-channel chunks

TRACE = False
LAST = {}
SCORES_BD = True  # scores via block-diagonal K (full 128-part contraction)
PIPELINE = 1      # emit attention this many T-tiles behind projections
STORE_SP = True   # issue output stores from SP instead of ACT
PROJBUFS = 3      # PSUM banks for projection groups
OPSBUFS = 1       # PSUM buffers per PV output tag (2 tags)
SPSBUFS = 1       # PSUM buffers per scores tag
ATTSPLIT = 1      # split exp/prob-mul for finer attention wavefront
PMAJOR = 1        # scores matmuls in window-parity-major order
ONBUFS = 2        # SBUF buffers for output staging tiles (2 tags)
SSPLIT = 1        # separate scores PSUM tile per window parity
S0DOUBLE = 1      # double-buffer parity-0 scores (uses the spare bank)
QEVAC_POOL = 0    # Q-projection evacuation on gpsimd instead of ACT
KEVAC_DVE = 0     # K block-diagonal evacuation on DVE instead of ACT
ABEFORE = 1       # emit attention subtile before (vs after) its slot
EBUFS = 2         # SBUF buffers for attention et/pt/on tiles
INTERLEAVE = 3    # quarter-T-shifted attention interleave (see driver)


def _emit(tc, out, xT, wq, wk, wv, ebt, bqk, iters=1, parts="pas"):
    """Emit the per-core program. bqk: [128, 8] fp32 (bq/8 | bk chunks) or None.
    parts: subset of 'p' (projections), 'a' (attention), 's' (store) for
    timing ablations."""
    nc = tc.nc
    Exp = mybir.ActivationFunctionType.Exp
    Ident = mybir.ActivationFunctionType.Identity

    with ExitStack() as ctx:
        ep = ctx.enter_context

        tbufs = PIPELINE + 1
        wpool = ep(tc.tile_pool(name="w", bufs=1))
        xpool = ep(tc.tile_pool(name="x", bufs=tbufs))
        qkpool = ep(tc.tile_pool(name="qk", bufs=tbufs))
        vpool = ep(tc.tile_pool(name="v", bufs=tbufs))
        epool = ep(tc.tile_pool(name="e", bufs=EBUFS))
        bdpool = ep(tc.tile_pool(name="bd", bufs=tbufs))
        rcpool = ep(tc.tile_pool(name="rc", bufs=4))
        onpool = ep(tc.tile_pool(name="on", bufs=ONBUFS))
        projps = ep(tc.tile_pool(name="projps", bufs=PROJBUFS, space="PSUM"))
        sps = ep(tc.tile_pool(name="sps", bufs=SPSBUFS, space="PSUM"))
        # one PSUM bank is spare: optionally double-buffer parity-0 scores
        sps0 = (ep(tc.tile_pool(name="sps0", bufs=2, space="PSUM"))
                if S0DOUBLE and SSPLIT else sps)
        ops = ep(tc.tile_pool(name="ops", bufs=OPSBUFS, space="PSUM"))

        # resident weights: [c_in chunk 128, c_out 512] fp16 per proj
        wsb = {}
        for nm, wdram in (("q", wq), ("k", wk), ("v", wv)):
            for ci in range(NCHUNK):
                t = wpool.tile([128, 512], F16, tag=f"w{nm}{ci}")
                nc.sync.dma_start(t[:], wdram[ci * 128:(ci + 1) * 128, :])
                wsb[nm, ci] = t
        ebt_sb = wpool.tile([128, 64], F16, tag="ebt")
        nc.sync.dma_start(ebt_sb[:], ebt[:, :])
        bqk_sb = None
        if bqk is not None:
            bqk_sb = wpool.tile([128, 8], F32, tag="bqk")
            nc.sync.dma_start(bqk_sb[:], bqk[:, :])

        # ---- one-time inits: zero blocks of block-diagonal tiles and the
        # ones-columns of V survive every iteration (later writes only
        # touch the data blocks), so initialize all rotating buffers here,
        # outside the timing loop. Tile calls advance each tag's rotation
        # by bufs=2, preserving in-loop phase.
        for i in range(max(tbufs, EBUFS)):
            if SCORES_BD and i < tbufs:
                for j in range(4):
                    t = bdpool.tile([128, 1024], F16, tag=f"bdk{j}")
                    nc.gpsimd.memset(t[:], 0)
            if i < EBUFS:
                t = epool.tile([128, 1024], F16, tag="pt")
                nc.gpsimd.memset(t[:], 0)
            if i < tbufs:
                for tt in range(NCHUNK):
                    vn = vpool.tile([128, 520], F16, tag=f"vn{tt}")
                    nc.scalar.activation(
                        vn[:].rearrange("p (h x) -> p h x", x=65)[:, :, 64],
                        ebt_sb[:, 0:8], Ident, bias=1.0, scale=0.0)

        if iters > 1:
            # on-device repetition for timing: amortizes host dispatch
            ep(tc.For_i(0, iters))

        def emit_proj_group(nm, co, xt, dst, bdk=None):
            """One projection PSUM group (4 matmuls) + its evacuation."""
            pi = 0 if nm == "q" else 1
            ps = projps.tile([128, 512], F32, tag="proj")
            if nm == "v":
                for ci in range(NCHUNK):
                    nc.tensor.matmul(
                        ps[:],
                        xt[ci][:, co * 128:(co + 1) * 128],
                        wsb["v", ci][:],
                        start=(ci == 0), stop=(ci == NCHUNK - 1))
                if "P" not in parts:
                    nc.vector.tensor_copy(
                        dst[:].rearrange("p (h x) -> p h x", x=65)
                        [:, :, 0:64],
                        ps[:].rearrange("p (h x) -> p h x", x=64))
                return
            for ci in range(NCHUNK):
                w_ap = (wsb[nm, 0][:, 0:128] if "W" in parts
                        else wsb[nm, ci][:, co * 128:(co + 1) * 128])
                if "A" in parts:  # ablation: independent matmuls, no accum
                    nc.tensor.matmul(
                        ps[:], w_ap, xt[0 if "W" in parts else ci][:],
                        start=True, stop=True)
                else:
                    nc.tensor.matmul(
                        ps[:], w_ap, xt[0 if "W" in parts else ci][:],
                        start=(ci == 0), stop=(ci == NCHUNK - 1))
            if "P" in parts:
                return
            if bdk is not None:
                # K straight into block-diagonal layout, window-major:
                # col g*128 + e*64 + k, g = 2tt+p
                bd_v = bdk[:].rearrange("r (g c) -> r g c", c=128)
                ps_v = ps[:].rearrange("r (g k) -> r g k", k=64)
                for e in range(2):
                    re = slice(e * 64, e * 64 + 64)
                    if bqk_sb is not None:
                        nc.scalar.activation(
                            bd_v[re, :, e * 64:e * 64 + 64],
                            ps_v[re], Ident,
                            bias=bqk_sb[re, 4 + co:5 + co])
                    elif KEVAC_DVE:
                        # unclog ACT's queue ahead of exp
                        nc.vector.tensor_copy(
                            bd_v[re, :, e * 64:e * 64 + 64], ps_v[re])
                    else:
                        nc.scalar.copy(
                            bd_v[re, :, e * 64:e * 64 + 64], ps_v[re])
            elif bqk_sb is not None:
                nc.scalar.activation(
                    dst[:], ps[:], Ident,
                    bias=bqk_sb[:, pi * 4 + co:pi * 4 + co + 1])
            elif QEVAC_POOL:
                # unclog ACT (whose queue delays exp): Q evac on gpsimd
                nc.gpsimd.tensor_copy(dst[:], ps[:])
            else:
                nc.scalar.copy(dst[:], ps[:])

        def proj_thunks(T):
            """xt loads (immediate) + 12 emission thunks for T's
            projection PSUM groups; returns (thunks, state_entry)."""
            xt = []
            for ci in range(NCHUNK):
                t = xpool.tile([128, 512], F16, tag=f"xt{ci}")
                nc.sync.dma_start(
                    t[:],
                    xT[ci * 128:(ci + 1) * 128, T * 512:(T + 1) * 512])
                xt.append(t)
            qkt = {}
            bdks = []
            vnat = []
            thunks = []

            def emit_proj_pair(nm, co0, xt):
                # ablation: interleave two groups' accumulation chains so
                # consecutive matmuls never hit the same PSUM region
                ps0 = projps.tile([128, 512], F32, tag="proj")
                ps1 = projps.tile([128, 512], F32, tag="proj")
                for ci in range(NCHUNK):
                    for k, ps in ((0, ps0), (1, ps1)):
                        co = co0 + k
                        nc.tensor.matmul(
                            ps[:],
                            wsb[nm, ci][:, co * 128:(co + 1) * 128],
                            xt[ci][:],
                            start=(ci == 0), stop=(ci == NCHUNK - 1))

            names = ("q",) if SCORES_BD else ("q", "k")
            for nm in names:
                if "I" in parts and "p" in parts:
                    for co0 in (0, 2):
                        thunks.append(
                            lambda nm=nm, co0=co0: emit_proj_pair(
                                nm, co0, xt))
                    for co in range(NCHUNK):
                        t = qkpool.tile([128, 512], F16, tag=f"{nm}t{co}")
                        qkt[nm, co] = t
                    continue
                for co in range(NCHUNK):
                    t = qkpool.tile([128, 512], F16, tag=f"{nm}t{co}")
                    qkt[nm, co] = t
                    if "p" in parts:
                        thunks.append(
                            lambda nm=nm, co=co, t=t: emit_proj_group(
                                nm, co, xt, t))
            if SCORES_BD:
                for j in range(4):
                    bdk = bdpool.tile([128, 1024], F16, tag=f"bdk{j}")
                    bdks.append(bdk)
                    if "p" in parts:
                        thunks.append(
                            lambda j=j, bdk=bdk: emit_proj_group(
                                "k", j, xt, None, bdk=bdk))
            for tt in range(NCHUNK):
                vn = vpool.tile([128, 520], F16, tag=f"vn{tt}")
                vnat.append(vn)
                if "p" in parts:
                    thunks.append(
                        lambda tt=tt, vn=vn: emit_proj_group(
                            "v", tt, xt, vn))
            return thunks, (qkt, bdks, vnat)

        def emit_attn_scores(qkt, bdks, Ta, tt):
            # ---- attention: subtile tt covers windows 2tt, 2tt+1 of Ta.
            # HAZARD RULE: concurrent matmuls with disjoint row-groups but
            # a shared column-group collide in the PE array (device crash);
            # sub-128 matmuls are placed DIAGONALLY (out partition base ==
            # operand partition base). Scores land head-parity packed (e on
            # halves); the exp(Bbias)-multiply on DVE moves probs to
            # block-diagonal window-parity layout, so PV runs full-width
            # against natural V and outputs land in natural token rows.
            if True:
                if SSPLIT and SCORES_BD:
                    # separate PSUM tile per window parity so exp of parity
                    # p depends on only its own 4 matmuls even with
                    # tile-granular PSUM dependency tracking
                    s2 = []
                    for p in range(2):
                        s_half = (sps0 if p == 0 else sps).tile(
                            [128, 256], F32, tag=f"s{p}")
                        s2.append(s_half)
                    for p in range(2):
                        for j in range(4):
                            w = 2 * tt + p
                            nc.tensor.matmul(
                                s2[p][:, j * 64:(j + 1) * 64],
                                bdks[j][:, tt * 256 + p * 128:
                                        tt * 256 + (p + 1) * 128],
                                qkt["q", j][:, w * 64:(w + 1) * 64],
                                start=True, stop=True)
                    s = None
                else:
                    s = sps.tile([128, 512], F32, tag="s")
                if SCORES_BD and not SSPLIT:
                    # p-major: with ATTSPLIT, exp of parity p waits only on
                    # its own 4 matmuls
                    for p, j in (
                            [(p, j) for p in range(2) for j in range(4)]
                            if PMAJOR else
                            [(p, j) for j in range(4) for p in range(2)]):
                        if True:
                            w = 2 * tt + p
                            nc.tensor.matmul(
                                s[:, (j * 2 + p) * 64:(j * 2 + p + 1) * 64],
                                bdks[j][:, tt * 256 + p * 128:
                                        tt * 256 + (p + 1) * 128],
                                qkt["q", j][:, w * 64:(w + 1) * 64],
                                start=True, stop=True)
                elif not SCORES_BD:
                    for j in range(4):
                        for e in range(2):
                            r = slice(e * 64, e * 64 + 64)
                            for p in range(2):
                                w = 2 * tt + p
                                wc = slice(w * 64, w * 64 + 64)
                                nc.tensor.matmul(
                                    s[r, (j * 2 + p) * 64:
                                      (j * 2 + p + 1) * 64],
                                    qkt["k", j][r, wc],
                                    qkt["q", j][r, wc],
                                    start=True, stop=True)
                et = epool.tile([128, 512], F16, tag="et")
                et_v = et[:].rearrange("r (j u q) -> r j u q", u=2, q=64)
                if SSPLIT and SCORES_BD:
                    for p in range(2):
                        nc.scalar.activation(
                            et_v[:, :, p, :],
                            s2[p][:].rearrange("r (j q) -> r j q", q=64),
                            Exp)
                elif ATTSPLIT:
                    # split exp per window parity: the (p, e) multiplies
                    # depend only on their own exp half
                    s_v = s[:].rearrange("r (j u q) -> r j u q", u=2, q=64)
                    for p in range(2):
                        nc.scalar.activation(
                            et_v[:, :, p, :], s_v[:, :, p, :], Exp)
                else:
                    nc.scalar.activation(et[:], s[:], Exp)
                # block-diagonal probs: pt[p*64+k, h*128+p*64+q] =
                # et[e*64+k, (2j+p)*64+q]*ebt[k,q] (h=2j+e); off-diagonal
                # blocks stay zero, so one PV matmul covers both windows
                # with full 128-partition contraction against natural V.
                pt = epool.tile([128, 1024], F16, tag="pt")
                pt_v = pt[:].rearrange("r (j z) -> r j z", j=4)
                for p in range(2):
                    rp = slice(p * 64, p * 64 + 64)
                    for e in range(2):
                        re = slice(e * 64, e * 64 + 64)
                        c0 = e * 128 + p * 64
                        if ATTSPLIT:
                            # split per bank-half: PV bank b waits only on
                            # the j in {2b, 2b+1} multiplies
                            for bh in range(2):
                                js = slice(bh * 2, bh * 2 + 2)
                                nc.vector.tensor_mul(
                                    pt_v[rp, js, c0:c0 + 64],
                                    et_v[re, js, p, :],
                                    ebt_sb[re, 0:64].unsqueeze(1)
                                    .broadcast_to((64, 2, 64)))
                        else:
                            nc.vector.tensor_mul(
                                pt_v[rp, :, c0:c0 + 64],
                                et_v[re, :, p, :],
                                ebt_sb[re, 0:64].unsqueeze(1)
                                .broadcast_to((64, 4, 64)))
                return pt

        def emit_attn_pv(pt, vnat, Ta, tt):
            if True:
                # PV: 8 matmuls (one per head), full 128 partitions; two
                # PSUM banks of 4 [128q2w, 65] units each.
                on = onpool.tile([128, 512], F32, tag=f"on{tt % 2}")
                for b in range(2):
                    o = ops.tile([128, 260], F32, tag=f"ob{b}")
                    o_v = o[:].rearrange("r (u x) -> r u x", x=65)
                    for u in range(4):
                        h = 4 * b + u
                        nc.tensor.matmul(
                            o[:, u * 65:(u + 1) * 65],
                            pt[:, h * 128:(h + 1) * 128],
                            vnat[tt][:, h * 65:(h + 1) * 65],
                            start=True, stop=True)
                    rc = rcpool.tile([128, 4], F32, tag=f"rc{b}")
                    nc.vector.reciprocal(rc[:, 0:4], o_v[:, :, 64])
                    nc.vector.tensor_mul(
                        on[:].rearrange("r (b2 u q) -> r b2 u q", b2=2, q=64)
                        [:, b, :, :],
                        o_v[:, :, 0:64],
                        rc[:, 0:4].unsqueeze(2).broadcast_to((128, 4, 64)))
                if "s" in parts:
                    eng = nc.sync if STORE_SP else nc.scalar
                    eng.dma_start(
                        out[Ta * 512 + tt * 128: Ta * 512 + (tt + 1) * 128,
                            :],
                        on[:])

        # software pipeline driver: attention trails projections by
        # PIPELINE T-tiles; with INTERLEAVE, attention subtiles are emitted
        # between projection groups as scheduler priority hints.
        state = {}
        for T in range(NT + PIPELINE):
            pthunks = []
            if T < NT:
                pthunks, entry = proj_thunks(T)
                state[T] = entry
            def attn_closures(Ta):
                q_, b_, v_ = state[Ta]
                cls = []
                for tt in range(NCHUNK):
                    cell = {}

                    def a_sc(tt=tt, q=q_, bb=b_, Ta=Ta, cell=cell):
                        cell["pt"] = emit_attn_scores(q, bb, Ta, tt)

                    def a_pv(tt=tt, v=v_, Ta=Ta, cell=cell):
                        emit_attn_pv(cell["pt"], v, Ta, tt)

                    if INTERLEAVE == 2:
                        cls.append((a_sc, a_pv))
                    else:
                        cls.append((lambda a=a_sc, b=a_pv: (a(), b()),))
                return cls

            if INTERLEAVE == 4 and "a" in parts:
                # all four subtiles inside their own T, each right after
                # the V-projection group it needs: no cross-T tail at all
                cls = list(attn_closures(T)) if T < NT else []
                for i, th in enumerate(pthunks):
                    th()
                    if i >= 8 and cls:
                        cls.pop(0)[0]()
                for tup in cls:
                    tup[0]()
                state.pop(T, None)
                continue

            if INTERLEAVE == 3 and "a" in parts:
                # quarter-T shift: tts 1-3 of T-1 go in proj(T) slots
                # {2,5,8}; tt 0 of T goes right after its first V group
                # (slot 8), shrinking the un-overlapped attention tail.
                if T < NT:
                    state[T] = (state[T][0], state[T][1], state[T][2],
                                attn_closures(T))
                athunks = list(state[T - 1][3][1:4]) if T >= 1 else []
                for i, th in enumerate(pthunks):
                    if ABEFORE and i in (2, 5, 7) and athunks:
                        athunks.pop(0)[0]()
                    th()
                    if not ABEFORE and i in (2, 5, 7) and athunks:
                        athunks.pop(0)[0]()
                    if i == 8 and T < NT:
                        state[T][3][0][0]()
                for th in athunks:
                    th[0]()
                if T >= 1:
                    state.pop(T - 1)
                continue

            athunks = []
            if T >= PIPELINE and "a" in parts:
                Ta = T - PIPELINE
                athunks = [t for t in attn_closures(Ta)]
                state.pop(Ta)
            if INTERLEAVE and pthunks and athunks:
                for i, th in enumerate(pthunks):
                    th()
                    if INTERLEAVE == 2:
                        if i % 3 != 0 and athunks:
                            a, b = athunks.pop(0)
                            a(), b()
                    elif i % 3 == 2 and athunks:
                        athunks.pop(0)[0]()
            else:
                for th in pthunks:
                    th()
            for tup in athunks:
                for f in tup:
                    f()


def _legalize_sync(nc, max_waits=1):
    """Hoist excess semaphore waits into standalone same-engine
    EventSemaphore instructions. Engine instruction streams execute in
    order, so a wait carried by an immediately-preceding EventSemaphore is
    equivalent to a wait on the instruction itself — and the walrus build
    in this environment rejects instructions with more than one wait."""
    import bass_rust
    n_new = 0
    fn = nc.m.functions[0]
    for blk in fn.blocks:
        out = []
        changed = False
        for ins in blk.instructions:
            si = ins.sync_info
            waits = list(si.on_wait) if si and si.on_wait else []
            if len(waits) > max_waits:
                keep = waits[-max_waits:]
                for w in waits[:-max_waits]:
                    es = mybir.InstEventSemaphore(
                        name=f"esw-{n_new}-{ins.name}", ins=[], outs=[])
                    es.engine = ins.engine
                    es.sync_info = bass_rust.SyncInfo(on_wait=[w], on_update=[])
                    out.append(es)
                    n_new += 1
                ins.sync_info = bass_rust.SyncInfo(
                    on_wait=keep,
                    on_update=list(si.on_update) if si.on_update else [])
                changed = True
            out.append(ins)
        if changed:
            blk.instructions = out
    return n_new


def _build_model(with_bias, iters=1, parts="pas"):
    nc = bass.Bass("TRN2", target_bir_lowering=False, debug=False,
                   enable_partition_id=False)
    xT = nc.dram_tensor("xT", [512, 4096], F16, kind="ExternalInput").ap()
    wq = nc.dram_tensor("wq", [512, 512], F16, kind="ExternalInput").ap()
    wk = nc.dram_tensor("wk", [512, 512], F16, kind="ExternalInput").ap()
    wv = nc.dram_tensor("wv", [512, 512], F16, kind="ExternalInput").ap()
    ebt = nc.dram_tensor("ebt", [128, 64], F16, kind="ExternalInput").ap()
    bqk = (nc.dram_tensor("bqk", [128, 8], F32, kind="ExternalInput").ap()
           if with_bias else None)
    out = nc.dram_tensor("out", [4096, 512], F32, kind="ExternalOutput").ap()
    with tile.TileContext(nc) as tc:
        _emit(tc, out, xT, wq, wk, wv, ebt, bqk, iters=iters, parts=parts)
    return nc


_MODEL_CACHE = {}


def get_model(with_bias=False, legalize=True, iters=1, parts="pas"):
    key = (with_bias, legalize, iters, parts, SCORES_BD, PIPELINE, STORE_SP,
           PROJBUFS, OPSBUFS, EBUFS, INTERLEAVE, SPSBUFS, ATTSPLIT, PMAJOR,
           ONBUFS, SSPLIT, S0DOUBLE, QEVAC_POOL, KEVAC_DVE, ABEFORE)
    if key not in _MODEL_CACHE:
        nc = _build_model(with_bias, iters=iters, parts=parts)
        if legalize:
            _legalize_sync(nc)
        _MODEL_CACHE[key] = nc
    return _MODEL_CACHE[key]


def make_in_maps(x, Wq, bq, Wk, bk, Wv, bv, Bbias):
    """Host-side sharding + layout prep. Returns (in_maps, with_bias)."""
    x = np.asarray(x, np.float32)
    with_bias = bool(np.any(bq) or np.any(bk))
    if np.any(bv):
        raise NotImplementedError("nonzero bv not supported")
    wq16 = np.ascontiguousarray(np.asarray(Wq, np.float32).T / 8.0).astype(np.float16)
    wk16 = np.ascontiguousarray(np.asarray(Wk, np.float32).T).astype(np.float16)
    wv16 = np.ascontiguousarray(np.asarray(Wv, np.float32).T).astype(np.float16)
    eb = np.exp(np.asarray(Bbias, np.float32).T)
    ebt = np.concatenate([eb, eb], 0).astype(np.float16)  # [128 (k x2), 64 q]
    common = {"wq": wq16, "wk": wk16, "wv": wv16, "ebt": ebt}
    if with_bias:
        bqk = np.concatenate(
            [np.asarray(bq, np.float32).reshape(4, 128).T / 8.0,
             np.asarray(bk, np.float32).reshape(4, 128).T], 1)  # [128, 8]
        common["bqk"] = np.ascontiguousarray(bqk)
    in_maps = []
    for b in range(B):
        xT16 = np.ascontiguousarray(
            x[b].reshape(TOK, C).T).astype(np.float16)
        in_maps.append({"xT": xT16, **common})
    return in_maps, with_bias


def kernel(**inputs):
    from concourse.bass_utils import run_bass_kernel_spmd
    in_maps, with_bias = make_in_maps(**inputs)
    nc = get_model(with_bias)
    res = run_bass_kernel_spmd(
        nc, in_maps, core_ids=list(range(B)), trace=TRACE)
    LAST["results"] = res
    out = np.stack([r["out"] for r in res.results], 0)
    return out.reshape(B, C, HH, WW)


def _harvest_io(nc):
    import jax
    in_names, out_names, out_avals = [], [], []
    for alloc in nc.m.functions[0].allocations:
        if not isinstance(alloc, mybir.MemoryLocationSet):
            continue
        name = alloc.memorylocations[0].name
        if alloc.kind == "ExternalInput":
            in_names.append(name)
        elif alloc.kind == "ExternalOutput":
            out_names.append(name)
            out_avals.append(jax.core.ShapedArray(
                tuple(alloc.tensor_shape), mybir.dt.np(alloc.dtype)))
    return in_names, out_names, out_avals


def _make_timed_callable(nc, in_maps):
    """Build a jitted shard_map callable around the single bass_exec of
    `nc` (mirrors run_bass_via_pjrt, but with NO donation so the same
    device-resident args can be reused across timed calls; outputs are
    garbage — timing only). Returns a zero-arg closure that runs one
    dispatch and blocks."""
    import jax
    from jax.sharding import Mesh, PartitionSpec
    from jax.experimental.shard_map import shard_map
    from concourse import bass2jax

    bass2jax.install_neuronx_cc_hook()
    in_names, out_names, out_avals = _harvest_io(nc)
    n_params = len(in_names)
    all_names = tuple(in_names + out_names)
    n_cores = len(in_maps)

    def _body(*args):
        return tuple(bass2jax._bass_exec_p.bind(
            *args,
            out_avals=tuple(out_avals),
            in_names=all_names,
            out_names=tuple(out_names),
            lowering_input_output_aliases=(),
            sim_require_finite=True,
            sim_require_nnan=True,
            nc=nc))

    devices = jax.devices()[:n_cores]
    mesh = Mesh(np.asarray(devices), ("core",))
    n_all = n_params + len(out_names)
    sharded = jax.jit(shard_map(
        _body, mesh=mesh,
        in_specs=(PartitionSpec("core"),) * n_all,
        out_specs=(PartitionSpec("core"),) * len(out_names),
        check_rep=False), keep_unused=True)
    concat_in = [
        np.concatenate([np.asarray(m[name]) for m in in_maps], 0)
        for name in in_names]
    concat_zeros = [
        np.zeros((n_cores * a.shape[0], *a.shape[1:]), a.dtype)
        for a in out_avals]
    args = [jax.device_put(a) for a in concat_in + concat_zeros]
    jax.block_until_ready(sharded(*args))  # warm-up / compile

    def run():
        jax.block_until_ready(sharded(*args))
    return run


def time_kernel(inputs, iters=4096, samples=8, parts="pas"):
    """Returns ns per iteration. Builds two model variants — the body run
    once vs `1+iters` times inside an on-device For_i loop — and
    differences median wall-clock over `samples` dispatches of each. With
    ~1s on-device per N-iter dispatch, the ~±20ms axon dispatch jitter
    contributes <2% error."""
    import time
    in_maps, with_bias = make_in_maps(**inputs)
    run1 = _make_timed_callable(
        get_model(with_bias, iters=1, parts=parts), in_maps)
    runN = _make_timed_callable(
        get_model(with_bias, iters=1 + iters, parts=parts), in_maps)
    t1s, tNs = [], []
    for _ in range(samples):
        t0 = time.time(); run1(); t1s.append(time.time() - t0)
        t0 = time.time(); runN(); tNs.append(time.time() - t0)
    t1 = float(np.median(t1s)); tN = float(np.median(tNs))
    return (tN - t1) / iters * 1e9, (t1s, tNs)

